# revision 18
# baseline (speedup 1.0000x reference)
"""Causal self-attention (S=2048, D=1024, 16 heads x 64) on 8 Trainium2 cores.

Tensor-parallel sharding: 2 heads per core. Each core computes
  qkv_local = x @ Wqkv[:, local]      (local q/k/v columns, q pre-scaled 1/8)
  attn_h    = softmax(mask(q_h k_h^T)) v_h          for its 2 heads
  partial   = concat(attn) @ Wout[local_rows, :]    (128 rows of Wout)
and the host sums the 8 partials (+bias).

All matmul operands are bf16 (fp32 PSUM accumulation); rel-err budget is
2e-2 and bf16 lands ~5e-3. The PE sustains ~0.43ns/moving-column only when
matmuls pipeline back-to-back (K=64 runs full rate as start/stop singles,
half rate in accumulation chains), so the schedule keeps the PE stream
dense:

 - logit pair-groups alternate heads, and each group's probs@v segment is
   issued one group later, so exp (ACT) and mask (DVE) of group k overlap
   the logit matmuls of group k+1 and the pv of group k-1
 - qkv production for the next 512-query block, v transposes, and the
   output projection of the previous block are ACT-free PE filler placed
   in the late group slots, where the final pv would otherwise wait on exp
 - diagonal blocks are narrowed to exactly the causal span; the masked
   region is always the first 128 columns of each diagonal block-slice,
   cut with one strided two-window multiply per group

On-chip layout: q^T/k^T are [128, S] with h0 on partitions 0-63 and h1 on
64-127; logit matmuls contract K=64 from the matching partition base (PE
tile rows 0 or 64). Logits are computed transposed ([key, query]) so
exp(logits) feeds probs@v directly as the moving operand; v carries an
appended ones-column so the same accumulation also produces the softmax
row-sums. No max-subtraction: logits are ~N(0,1) after the 1/8 scale, well
within fp32 exp range; masked entries are zeroed after exp.
"""

import numpy as np

import concourse.bass as bass
import concourse.mybir as mybir
import concourse.tile as tile
from concourse import bacc
from concourse.bass_utils import run_bass_kernel_spmd

S = 2048
D = 1024
DH = 64
N_CORES = 8

P = 128
NB512 = S // 512  # 512-wide query chunks
NB128 = S // 128  # 128-wide chunks
KO = D // P  # contraction chunks for the projections

F32 = mybir.dt.float32
BF16 = mybir.dt.bfloat16

_compiled = {}


def _groups(ic):
    """Logit pair-groups for query block ic: [(jc, col_start, n, i0), ...]
    per group. i0/n give the exact causal query span of key-block jc."""
    gs = []
    for jp in range(2 * ic):
        gs.append([(2 * jp, 0, 512, 0), (2 * jp + 1, 512, 512, 0)])
    gs.append([(4 * ic, 0, 512, 0), (4 * ic + 1, 512, 384, 128)])
    gs.append([(4 * ic + 2, 0, 256, 256), (4 * ic + 3, 256, 128, 384)])
    return gs


def _emit(nc, tc, mm_dt, out_dt, xt, w, wout, maskt, ident, out):
    f32 = F32
    with (
        tc.tile_pool(name="const", bufs=1) as const,
        tc.tile_pool(name="epool", bufs=6) as epool,
        tc.tile_pool(name="opool", bufs=4) as opool,
        tc.tile_pool(name="rcpool", bufs=1) as rcpool,
        tc.tile_pool(name="pslog", bufs=2, space="PSUM") as pslog,
        tc.tile_pool(name="psqk", bufs=2, space="PSUM") as psqk,
        tc.tile_pool(name="psacc", bufs=2, space="PSUM") as psacc,
    ):
        sb_xT = const.tile([P, KO, S], mm_dt, name="sb_xT")
        sb_w = const.tile([P, KO, 384], mm_dt, name="sb_w")
        sb_wout = const.tile([P, D], mm_dt, name="sb_wout")
        sb_mask = const.tile([P, 2, P], mm_dt, name="sb_mask")
        sb_qT = const.tile([P, S], mm_dt, name="sb_qT")  # h0 rows 0-63, h1 64-127
        sb_kT = const.tile([P, S], mm_dt, name="sb_kT")
        sb_v = const.tile([P, NB128, 130], mm_dt, name="sb_v")
        sb_vT = const.tile([P, S], mm_dt, name="sb_vT")
        sb_attnT = const.tile([P, S], mm_dt, name="sb_attnT")
        sb_ident = const.tile([P, P], mm_dt, name="sb_ident")

        # loads: small weights on the GpSimd SWDGE queue; xT [o, si] chunks
        # are host-tiled so each DMA is one contiguous 128KB block, streamed
        # si-major across the two HWDGE queues (sync/scalar) so the first
        # q/k chunk arrives as early as possible
        for o in range(KO):
            nc.gpsimd.dma_start(sb_w[:, o, :], w[o * P : (o + 1) * P, :])
        for si in range(NB512):
            sl = slice(si * 512, (si + 1) * 512)
            for o in range(KO):
                weng = nc.sync if o % 2 == 0 else nc.scalar
                weng.dma_start(sb_xT[:, o, sl], xt[o, si])
        nc.gpsimd.dma_start(sb_mask[:], maskt[:])
        nc.gpsimd.dma_start(sb_wout[:], wout[:])
        nc.gpsimd.dma_start(sb_ident[:], ident[:])

        def _ms(eng, ap, val):
            # f32r tiles cannot be memset directly; reinterpret as f32
            eng.memset(ap.bitcast(F32) if mm_dt == mybir.dt.float32r else ap, val)

        _ms(nc.gpsimd, sb_v[:, :, DH], 1.0)
        _ms(nc.gpsimd, sb_v[:, :, 129], 1.0)

        # q^T / k^T producer: [c, s] = sum_D W[D, c] * xT[D, s]
        def emit_qk(si):
            for cc, dest in ((0, sb_qT), (1, sb_kT)):
                ps = psqk.tile([P, 512], f32, name="ps_qk", tag="mm")
                for o in range(KO):
                    nc.tensor.matmul(
                        ps[:],
                        sb_w[:, o, cc * P : (cc + 1) * P],
                        sb_xT[:, o, si * 512 : (si + 1) * 512],
                        start=(o == 0),
                        stop=(o == KO - 1),
                    )
                nc.vector.tensor_copy(
                    dest[:, si * 512 : (si + 1) * 512], ps[:]
                )

        # v^T producer (same efficient N=512 shape as q/k), then PE-mode
        # transposes turn each 128x128 block into v natural layout
        def emit_vT(si):
            psv = psqk.tile([P, 512], f32, name="ps_vT", tag="mm")
            for o in range(KO):
                nc.tensor.matmul(
                    psv[:],
                    sb_w[:, o, 256:384],
                    sb_xT[:, o, si * 512 : (si + 1) * 512],
                    start=(o == 0),
                    stop=(o == KO - 1),
                )
            nc.vector.tensor_copy(sb_vT[:, si * 512 : (si + 1) * 512], psv[:])

        def emit_v(sc):
            pt = psqk.tile([P, P], mm_dt, name="ps_t", tag="mm")
            nc.tensor.transpose(
                pt[:], sb_vT[:, sc * P : (sc + 1) * P], sb_ident[:]
            )
            # single strided copy: [h, dh] segments land at cols h*65
            dst = sb_v[:, sc, 0:130].rearrange("p (h c) -> p h c", h=2)[:, :, 0:DH]
            src = pt[:, 0:P].rearrange("p (h c) -> p h c", h=2)
            nc.vector.tensor_copy(dst, src)

        # output projection for one 128-row query chunk
        def emit_proj(sc):
            for ec in range(D // 512):
                pp = psqk.tile([P, 512], f32, name="ps_p", tag="mm")
                nc.tensor.matmul(
                    pp[:],
                    sb_attnT[:, sc * P : (sc + 1) * P],
                    sb_wout[:, ec * 512 : (ec + 1) * 512],
                    start=True,
                    stop=True,
                )
                ot = opool.tile([P, 512], out_dt, name="ot", tag="ot")
                # keep the exp stream unblocked: ACT only takes the copies
                # that land while attention is still shallow (ic=0's blocks)
                if sc < 2:
                    nc.scalar.copy(ot[:], pp[:])
                else:
                    nc.vector.tensor_copy(ot[:], pp[:])
                oeng = nc.sync if (sc * 2 + ec) % 2 == 0 else nc.gpsimd
                oeng.dma_start(out[ec, sc], ot[:])

        # one logit pair-group: 2 K=64 single matmuls -> exp -> strided
        # two-window triangle mask (the masked span of each diagonal block
        # is always its first 128 slice columns)
        def emit_L(ic, h, grp):
            hp = slice(h * DH, (h + 1) * DH)
            tot = grp[-1][1] + grp[-1][2]
            pl = pslog.tile([P, 1024], f32, name="ps_l", tag="mm2")
            for jc, cs, n, i0 in grp:
                nc.tensor.matmul(
                    pl[:, cs : cs + n],
                    sb_kT[hp, jc * P : (jc + 1) * P],
                    sb_qT[hp, ic * 512 + i0 : ic * 512 + i0 + n],
                    start=True,
                    stop=True,
                )
            e = epool.tile([P, 1024], mm_dt, name="e_t", tag="e")
            nc.scalar.activation(
                e[:, :tot], pl[:, :tot], mybir.ActivationFunctionType.Exp
            )
            if grp[0][0] >= 4 * ic:  # diagonal group: both windows, one mul
                stride = grp[1][1]  # 512 for group A, 256 for group B
                ev = e[:, 0 : 2 * stride].rearrange(
                    "p (g c) -> p g c", g=2
                )[:, :, 0:P]
                nc.vector.tensor_mul(ev, ev, sb_mask[:])
            return e

        # probs@v segment for one pair-group (accumulates into acc)
        def emit_pv(acc, h, e, grp, start, stop):
            last = grp[-1]
            for jc, cs, n, i0 in grp:
                nc.tensor.matmul(
                    acc[:, i0 : i0 + n],
                    sb_v[:, jc, h * 65 : (h + 1) * 65],
                    e[:, cs : cs + n],
                    start=start and (jc == grp[0][0]),
                    stop=stop and (jc == last[0]),
                )

        # normalize: reciprocal of the rowsum row, broadcast across
        # partitions on the (otherwise idle) GpSimd engine, then one
        # PSUM-reading multiply straight into attnT
        def emit_norm(ic, h, acc):
            po = h * DH
            rsk = rcpool.tile([1, 512], f32, name="rsk", tag="rsk", bufs=2)
            nc.scalar.copy(rsk[:], acc[DH : DH + 1, :])
            rck = rcpool.tile([1, 512], f32, name="rck", tag="rck", bufs=3)
            nc.vector.reciprocal_approx_fast(rck[:], rsk[:])
            bck = rcpool.tile([DH, 512], f32, name="bck", tag="bck", bufs=3)
            nc.gpsimd.partition_broadcast(bck[:], rck[:])
            dst = sb_attnT[po : po + DH, ic * 512 : (ic + 1) * 512]
            nc.vector.tensor_mul(dst, acc[0:DH, :], bck[:])

        # ---- schedule ----
        emit_qk(0)

        for ic in range(NB512):
            gs = _groups(ic)
            G = len(gs)
            # ACT-free PE filler for this iteration: v production for this
            # block (pv needs it), next block's q/k/v, previous block's
            # output projection
            fillers = []
            if ic == 0:
                fillers.append(lambda: emit_vT(0))
                for sc in range(4):
                    fillers.append(lambda sc=sc: emit_v(sc))
            if ic + 1 < NB512:
                fillers.append(lambda ic=ic: emit_qk(ic + 1))
                fillers.append(lambda ic=ic: emit_vT(ic + 1))
                for sc in range(4 * (ic + 1), 4 * (ic + 2)):
                    fillers.append(lambda sc=sc: emit_v(sc))
            if ic > 0:
                for sc in range(4 * (ic - 1), 4 * ic):
                    fillers.append(lambda sc=sc: emit_proj(sc))

            accs = {
                h: psacc.tile([DH + 1, 512], f32, name=f"ps_acc{h}", tag="acc")
                for h in (0, 1)
            }
            es = {}
            # pv for group k-1 is issued behind the logits of group k, so
            # exp/mask always have a full group of PE time to complete.
            # Fillers go to the LATE slots so the final pv + norm + proj
            # tail always has ACT-free PE work in front of it.
            for k in range(G):
                for h in (0, 1):
                    es[(h, k)] = emit_L(ic, h, gs[k])
                    if k >= 1:
                        emit_pv(accs[h], h, es[(h, k - 1)], gs[k - 1],
                                start=(k == 1), stop=False)
                if ic == 0 and k == 0:
                    # all four v blocks must exist before the first pv
                    for _ in range(5):
                        fillers.pop(0)()
                elif G - 1 - k < len(fillers):
                    fillers.pop(0)()
            for h in (0, 1):
                emit_pv(accs[h], h, es[(h, G - 1)], gs[G - 1],
                        start=(G == 1), stop=True)
                emit_norm(ic, h, accs[h])
            for fn in fillers:
                fn()

        for sc in range(4 * (NB512 - 1), 4 * NB512):
            emit_proj(sc)


def build(mm_dt=BF16):
    key = str(mm_dt)
    if key in _compiled:
        return _compiled[key]
    out_dt = BF16 if mm_dt == BF16 else F32
    nc = bacc.Bacc("TRN2", target_bir_lowering=False, debug=False, num_devices=N_CORES)
    # xt/out are host-tiled so every DMA block is one contiguous 128KB run
    xt = nc.dram_tensor("xt", [KO, NB512, P, 512], mm_dt, kind="ExternalInput").ap()
    w = nc.dram_tensor("w", [D, 384], mm_dt, kind="ExternalInput").ap()
    wout = nc.dram_tensor("wout", [P, D], mm_dt, kind="ExternalInput").ap()
    maskt = nc.dram_tensor("maskt", [P, 2, P], mm_dt, kind="ExternalInput").ap()
    ident = nc.dram_tensor("ident", [P, P], mm_dt, kind="ExternalInput").ap()
    out = nc.dram_tensor("out", [2, NB128, P, 512], out_dt, kind="ExternalOutput").ap()
    with tile.TileContext(nc) as tc:
        _emit(nc, tc, mm_dt, out_dt, xt, w, wout, maskt, ident, out)
    nc.compile()
    _compiled[key] = nc
    return nc


def make_inputs(x, Wqkv, Wout, mm_dt=BF16):
    """Host-side shard/layout prep -> per-core input maps."""
    np_dt = mybir.dt.np(mm_dt)
    x = np.ascontiguousarray(np.asarray(x, np.float32))
    Wqkv = np.asarray(Wqkv, np.float32)
    Wout = np.asarray(Wout, np.float32)
    # [D, S] -> [o, si, p, 512] so each (o, si) chunk is contiguous
    xT = np.ascontiguousarray(
        x.T.reshape(KO, P, NB512, 512).transpose(0, 2, 1, 3)
    ).astype(np_dt)
    j = np.arange(P, dtype=np.int64)
    tri = (j[:, None] <= j[None, :]).astype(np.float32)  # p <= c
    mask = np.ascontiguousarray(
        np.broadcast_to(tri[:, None, :], (P, 2, P))
    ).astype(np_dt)
    ident = np.eye(P, dtype=np.float32).astype(np_dt)
    in_maps = []
    for c in range(N_CORES):
        wq = Wqkv[:, 128 * c : 128 * (c + 1)] * (1.0 / np.sqrt(DH))
        wk = Wqkv[:, D + 128 * c : D + 128 * (c + 1)]
        wv = Wqkv[:, 2 * D + 128 * c : 2 * D + 128 * (c + 1)]
        w_loc = np.ascontiguousarray(np.concatenate([wq, wk, wv], axis=1))
        wout_loc = np.ascontiguousarray(Wout[128 * c : 128 * (c + 1), :])
        in_maps.append(
            {
                "xt": xT,
                "w": w_loc.astype(np_dt),
                "wout": wout_loc.astype(np_dt),
                "maskt": mask,
                "ident": ident,
            }
        )
    return in_maps


def kernel(x, Wqkv, Wout, bias, mm_dt=BF16, **run_kwargs):
    nc = build(mm_dt)
    in_maps = make_inputs(x, Wqkv, Wout, mm_dt)
    res = run_bass_kernel_spmd(nc, in_maps, core_ids=list(range(N_CORES)), **run_kwargs)
    acc = np.zeros((S, D), np.float64)
    for c in range(N_CORES):
        o4 = res.results[c]["out"].astype(np.float64)  # [ec, sc, p, 512]
        acc += o4.transpose(1, 2, 0, 3).reshape(S, D)
    acc += np.asarray(bias, np.float64)[None, :]
    return acc.astype(np.float32)
